# revision 1
# baseline (speedup 1.0000x reference)
"""Trainium2 Bass kernel for nn_ConvBlock (Chebyshev graph conv + BatchNorm + ReLU).

Sharding: data-parallel over batch (B=8 -> 1 sample per NeuronCore).
Per core: Chebyshev recursion via indirect-DMA row gathers + PE scatter-matmuls
(edge weights folded into host-built one-hot scatter blocks), K-stacked GEMM with
host-folded power-basis weights, BatchNorm stats on device (combined across cores
on host between two launches), normalize+ReLU+transpose on device.
"""
import os, sys
sys.path.insert(0, '/opt/trn_rl_repo')
import numpy as np
from contextlib import ExitStack

import concourse.bass as bass
import concourse.tile as tile
from concourse import bacc, mybir
from concourse.bass_utils import run_bass_kernel_spmd
from concourse.masks import make_identity

B, V, E = 8, 12288, 98304
FIN, FOUT, K = 256, 256, 4
EPS = 1e-5
P = 128
GSZ = 64            # dst-group node window (S_w block width)
NVT = V // P        # 96 vtiles (group pairs)
NCH = 24            # GEMM chunks of 512 nodes
CHV = NVT // NCH    # 4 vtiles per chunk

F32 = mybir.dt.float32
AF = mybir.ActivationFunctionType

_cache = {}


def _build_schedule(edge_src, edge_dst, edge_weight):
    """Group edges by 64-node dst windows, pad each group to multiples of 128."""
    g_of_e = edge_dst // GSZ
    order = np.argsort(g_of_e, kind='stable')
    NG = V // GSZ
    counts = np.bincount(g_of_e, minlength=NG)
    sub_of_g = np.maximum(1, (counts + P - 1) // P)   # subtiles per group
    ST = int(sub_of_g.sum())
    idx_np = np.zeros((ST, P), np.int32)              # src per (subtile, lane)
    sw = np.zeros((ST, P, GSZ), np.float32)           # scatter blocks
    vt_subs = [[] for _ in range(NVT)]                # subtile ids per vtile half
    t = 0
    pos = 0
    for g in range(NG):
        eg = order[pos:pos + counts[g]]
        pos += counts[g]
        for s in range(sub_of_g[g]):
            part = eg[s * P:(s + 1) * P]
            n = len(part)
            idx_np[t, :n] = edge_src[part]
            sw[t, np.arange(n), edge_dst[part] - g * GSZ] = edge_weight[part]
            vt_subs[g // 2].append((t, g % 2))
            t += 1
    assert t == ST
    return idx_np, sw, vt_subs, ST


def _fold_weights(weight):
    # out = sum_k T_k(L) x W_k ; T0=I, T1=L, T2=2L^2-1, T3=4L^3-3L
    # power basis z_j = L^j x :  out = sum_j z_j Wf_j
    W = weight
    Wf = np.stack([W[0] - W[2], W[1] - 3.0 * W[3], 2.0 * W[2], 4.0 * W[3]])
    # [(j,i), o] -> tiles [8, 128, 256]
    return Wf.reshape(K * FIN, FOUT).reshape(8, P, FOUT).copy()


def _build_launch_a(ST, vt_subs):
    nc = bacc.Bacc("TRN2", target_bir_lowering=False, debug=False, num_devices=8)
    xb = nc.dram_tensor("xb", [V, FIN], F32, kind="ExternalInput").ap()
    msg0 = nc.dram_tensor("msg0", [P, ST, FIN], F32, kind="ExternalInput").ap()
    idx = nc.dram_tensor("idx", [P, ST], mybir.dt.int32, kind="ExternalInput").ap()
    swt = nc.dram_tensor("swt", [P, ST * GSZ], F32, kind="ExternalInput").ap()
    wf = nc.dram_tensor("wf", [8, P, FOUT], F32, kind="ExternalInput").ap()
    rawT = nc.dram_tensor("rawT", [2, P, V], F32, kind="ExternalOutput").ap()
    stats = nc.dram_tensor("stats", [P, 4], F32, kind="ExternalOutput").ap()
    zd = [xb] + [nc.dram_tensor(f"z{j}", [V, FIN], F32).ap() for j in (1, 2, 3)]

    with tile.TileContext(nc) as tc, ExitStack() as ctx:
        cpool = ctx.enter_context(tc.tile_pool(name="const", bufs=1))
        idx_t = cpool.tile([P, ST], mybir.dt.int32, tag="idx")
        nc.sync.dma_start(idx_t[:], idx[:, :])
        ident = cpool.tile([P, P], F32, tag="id")
        make_identity(nc, ident[:])
        wf_t = cpool.tile([P, 8 * FOUT], F32, tag="wf")
        nc.sync.dma_start(wf_t[:].rearrange("p (k o) -> p k o", k=8), wf.transpose([1, 0, 2]))

        # ---- Chebyshev (power-basis) recursion: z_j = L z_{j-1} ----
        with ExitStack() as rctx:
            swp = rctx.enter_context(tc.tile_pool(name="swp", bufs=3))
            msgp = rctx.enter_context(tc.tile_pool(name="msgp", bufs=3))
            outp = rctx.enter_context(tc.tile_pool(name="outp", bufs=3))
            psp = rctx.enter_context(tc.tile_pool(name="psp", bufs=4, space="PSUM"))
            for j in (1, 2, 3):
                for vt in range(NVT):
                    subs = vt_subs[vt]
                    nst = len(subs)
                    t0 = subs[0][0]
                    sw_t = swp.tile([P, nst * GSZ], F32, tag="sw")
                    nc.sync.dma_start(sw_t[:], swt[:, t0 * GSZ:(t0 + nst) * GSZ])
                    msg_t = msgp.tile([P, nst * FIN], F32, tag="msg")
                    if j == 1:
                        nc.sync.dma_start(
                            msg_t[:].rearrange("p (t f) -> p t f", t=nst),
                            msg0[:, t0:t0 + nst, :])
                    else:
                        for s in range(nst):
                            nc.gpsimd.indirect_dma_start(
                                out=msg_t[:, s * FIN:(s + 1) * FIN], out_offset=None,
                                in_=zd[j - 1][:, :],
                                in_offset=bass.IndirectOffsetOnAxis(
                                    ap=idx_t[:, t0 + s:t0 + s + 1], axis=0))
                    ps = psp.tile([P, FIN], F32, tag="acc")
                    half_count = [sum(1 for _, h in subs if h == hh) for hh in (0, 1)]
                    seen = [0, 0]
                    for s, (t, h) in enumerate(subs):
                        nc.tensor.matmul(
                            ps[h * GSZ:(h + 1) * GSZ, :],
                            sw_t[:, s * GSZ:(s + 1) * GSZ],
                            msg_t[:, s * FIN:(s + 1) * FIN],
                            start=(seen[h] == 0), stop=(seen[h] == half_count[h] - 1))
                        seen[h] += 1
                    o_t = outp.tile([P, FIN], F32, tag="zo")
                    nc.scalar.activation(o_t[:], ps[:], AF.Copy)
                    nc.sync.dma_start(
                        zd[j].rearrange("(vt p) f -> vt p f", p=P)[vt], o_t[:])

        # ---- GEMM + BN stats ----
        with ExitStack() as gctx:
            zin = gctx.enter_context(tc.tile_pool(name="zin", bufs=2))
            ztp = gctx.enter_context(tc.tile_pool(name="ztp", bufs=2))
            big = gctx.enter_context(tc.tile_pool(name="big", bufs=1))
            psT = gctx.enter_context(tc.tile_pool(name="psT", bufs=4, space="PSUM"))
            psG = gctx.enter_context(tc.tile_pool(name="psG", bufs=2, space="PSUM"))
            rawT_sb = big.tile([P, 2 * V], F32, tag="rawT")
            stats_sb = big.tile([P, 2 * NCH * 6], F32, tag="stats")
            for c in range(NCH):
                zin_t = zin.tile([P, K * CHV * FIN], F32, tag="zin")
                for j in range(K):
                    nc.sync.dma_start(
                        zin_t[:, j * CHV * FIN:(j + 1) * CHV * FIN]
                        .rearrange("p (vt f) -> p vt f", vt=CHV),
                        zd[j].rearrange("(c vt p) f -> c p vt f", vt=CHV, p=P)[c])
                zT = ztp.tile([P, 8 * 512], F32, tag="zT")
                for j in range(K):
                    for vt in range(CHV):
                        for fh in range(2):
                            pt = psT.tile([P, P], F32, tag="pt")
                            nc.tensor.transpose(
                                pt[:],
                                zin_t[:, (j * CHV + vt) * FIN + fh * P:
                                      (j * CHV + vt) * FIN + fh * P + P],
                                ident[:])
                            kt = j * 2 + fh
                            eng = nc.vector if (vt + fh) % 2 == 0 else nc.scalar
                            if eng is nc.vector:
                                nc.vector.tensor_copy(zT[:, kt * 512 + vt * P: kt * 512 + vt * P + P], pt[:])
                            else:
                                nc.scalar.activation(zT[:, kt * 512 + vt * P: kt * 512 + vt * P + P], pt[:], AF.Copy)
                for oh in range(2):
                    pg = psG.tile([P, 512], F32, tag="pg")
                    for kt in range(8):
                        nc.tensor.matmul(
                            pg[:], wf_t[:, kt * FOUT + oh * P: kt * FOUT + oh * P + P],
                            zT[:, kt * 512:(kt + 1) * 512],
                            start=(kt == 0), stop=(kt == 7))
                    nc.vector.bn_stats(stats_sb[:, (oh * NCH + c) * 6:(oh * NCH + c) * 6 + 6], pg[:])
                    nc.scalar.activation(rawT_sb[:, oh * V + c * 512: oh * V + (c + 1) * 512], pg[:], AF.Copy)
            for oh in range(2):
                nc.sync.dma_start(rawT[oh], rawT_sb[:, oh * V:(oh + 1) * V])
            aggr = big.tile([P, 4], F32, tag="aggr")
            for oh in range(2):
                nc.vector.bn_aggr(aggr[:, oh * 2:oh * 2 + 2],
                                  stats_sb[:, oh * NCH * 6:(oh + 1) * NCH * 6])
            # stats out: [mean_h0, ex2_h0, mean_h1, ex2_h1]
            so = big.tile([P, 4], F32, tag="so")
            for oh in range(2):
                m = aggr[:, oh * 2:oh * 2 + 1]
                v_ = aggr[:, oh * 2 + 1:oh * 2 + 2]
                nc.vector.tensor_copy(so[:, oh * 2:oh * 2 + 1], m)
                nc.vector.tensor_tensor(out=so[:, oh * 2 + 1:oh * 2 + 2], in0=m, in1=m,
                                        op=mybir.AluOpType.mult)
                nc.vector.tensor_tensor(out=so[:, oh * 2 + 1:oh * 2 + 2],
                                        in0=so[:, oh * 2 + 1:oh * 2 + 2], in1=v_,
                                        op=mybir.AluOpType.add)
            nc.sync.dma_start(stats[:, :], so[:])
    nc.compile()
    return nc


def _build_launch_b():
    nc = bacc.Bacc("TRN2", target_bir_lowering=False, debug=False, num_devices=8)
    rawT = nc.dram_tensor("rawT", [2, P, V], F32, kind="ExternalInput").ap()
    sc = nc.dram_tensor("sc", [P, 2], F32, kind="ExternalInput").ap()
    sh = nc.dram_tensor("sh", [P, 2], F32, kind="ExternalInput").ap()
    out = nc.dram_tensor("out", [V, FOUT], F32, kind="ExternalOutput").ap()
    CH2 = 8           # vtiles per chunk
    NC2 = NVT // CH2  # 12 chunks
    with tile.TileContext(nc) as tc, ExitStack() as ctx:
        cpool = ctx.enter_context(tc.tile_pool(name="const", bufs=1))
        ident = cpool.tile([P, P], F32, tag="id")
        make_identity(nc, ident[:])
        sc_t = cpool.tile([P, 2], F32, tag="sc")
        sh_t = cpool.tile([P, 2], F32, tag="sh")
        nc.sync.dma_start(sc_t[:], sc[:, :])
        nc.sync.dma_start(sh_t[:], sh[:, :])
        pool = ctx.enter_context(tc.tile_pool(name="sb", bufs=2))
        psp = ctx.enter_context(tc.tile_pool(name="ps", bufs=4, space="PSUM"))
        for c in range(NC2):
            nt = pool.tile([P, 2 * CH2 * P], F32, tag="nt")
            for oh in range(2):
                nc.sync.dma_start(nt[:, oh * CH2 * P:(oh + 1) * CH2 * P],
                                  rawT[oh][:, c * CH2 * P:(c + 1) * CH2 * P])
            for oh in range(2):
                nc.scalar.activation(
                    nt[:, oh * CH2 * P:(oh + 1) * CH2 * P],
                    nt[:, oh * CH2 * P:(oh + 1) * CH2 * P],
                    AF.Relu, bias=sh_t[:, oh:oh + 1], scale=sc_t[:, oh:oh + 1])
            ot = pool.tile([P, CH2 * FOUT], F32, tag="ot")
            for vt in range(CH2):
                pt = psp.tile([P, FOUT], F32, tag="pt")
                for oh in range(2):
                    nc.tensor.transpose(
                        pt[:, oh * P:(oh + 1) * P],
                        nt[:, oh * CH2 * P + vt * P: oh * CH2 * P + (vt + 1) * P],
                        ident[:])
                eng = vt % 2
                if eng == 0:
                    nc.vector.tensor_copy(ot[:, vt * FOUT:(vt + 1) * FOUT], pt[:])
                else:
                    nc.scalar.activation(ot[:, vt * FOUT:(vt + 1) * FOUT], pt[:], AF.Copy)
            nc.sync.dma_start(
                out.rearrange("(c vt p) f -> c p vt f", vt=CH2, p=P)[c],
                ot[:].rearrange("p (vt f) -> p vt f", vt=CH2))
    nc.compile()
    return nc


def kernel(x, edge_weight, weight, bias, gamma, beta, edge_src, edge_dst):
    x = np.asarray(x, np.float32)
    edge_weight = np.asarray(edge_weight, np.float32)
    weight = np.asarray(weight, np.float32)
    gamma = np.asarray(gamma, np.float32)
    beta = np.asarray(beta, np.float32)
    edge_src = np.asarray(edge_src, np.int32)
    edge_dst = np.asarray(edge_dst, np.int32)

    idx_np, sw, vt_subs, ST = _build_schedule(edge_src, edge_dst, edge_weight)
    key = ("A", ST, tuple(len(s) for s in vt_subs))
    if key not in _cache:
        _cache[key] = _build_launch_a(ST, vt_subs)
    ncA = _cache[key]
    if "B" not in _cache:
        _cache["B"] = _build_launch_b()
    ncB = _cache["B"]

    wf = _fold_weights(weight)
    swt = np.ascontiguousarray(sw.transpose(1, 0, 2)).reshape(P, ST * GSZ)
    idx_t = np.ascontiguousarray(idx_np.T)             # [P, ST]
    in_maps = []
    for b in range(B):
        msg0 = x[b][idx_np.reshape(-1)].reshape(ST, P, FIN).transpose(1, 0, 2)
        in_maps.append({
            "xb": np.ascontiguousarray(x[b]),
            "msg0": np.ascontiguousarray(msg0),
            "idx": idx_t, "swt": swt, "wf": wf,
        })
    resA = run_bass_kernel_spmd(ncA, in_maps, core_ids=list(range(B)))

    # host: combine BN stats across cores (equal counts -> simple average)
    st = np.stack([resA.results[b]["stats"] for b in range(B)])   # [B, 128, 4]
    mean = st[:, :, [0, 2]].mean(0)                               # [128, 2]
    ex2 = st[:, :, [1, 3]].mean(0)
    var = ex2 - mean * mean
    g2 = gamma.reshape(2, P).T                                    # [128, 2]
    b2 = beta.reshape(2, P).T
    scale = (g2 / np.sqrt(var + EPS)).astype(np.float32)
    shift = (b2 - mean * scale).astype(np.float32)

    in_maps_b = [{"rawT": resA.results[b]["rawT"], "sc": scale, "sh": shift}
                 for b in range(B)]
    resB = run_bass_kernel_spmd(ncB, in_maps_b, core_ids=list(range(B)))
    global _last_inmaps
    _last_inmaps = {key: in_maps, "B": in_maps_b}
    out = np.stack([resB.results[b]["out"] for b in range(B)])
    # bias cancels inside training-mode BN (shifts the mean only); gamma/beta applied above
    return out.astype(np.float32)



# revision 7
# speedup vs baseline: 3.0694x; 3.0694x over previous
"""Trainium2 Bass kernel for nn_ConvBlock (Chebyshev graph conv + BatchNorm + ReLU).

Sharding: data-parallel over batch (B=8 -> 1 sample per NeuronCore).
Per core: power-basis Chebyshev recursion z_j = L z_{j-1} via batched
indirect-DMA row gathers (bf16) + PE scatter-matmuls (edge weights folded into
host-built one-hot scatter blocks), K-stacked GEMM with host-folded power-basis
weights consuming z^T via DMA-transpose loads, BatchNorm stats on device
(combined across cores on host between two launches), normalize+ReLU+transpose
on device. All matmul operands bf16 (fp32 PSUM accumulation).
"""
import os, sys
sys.path.insert(0, '/opt/trn_rl_repo')
import numpy as np
import ml_dtypes
from contextlib import ExitStack

import concourse.bass as bass
import concourse.tile as tile
from concourse import bacc, mybir
from concourse.bass_utils import run_bass_kernel_spmd

B, V, E = 8, 12288, 98304
FIN, FOUT, K = 256, 256, 4
EPS = 1e-5
P = 128
GSZ = 64            # dst-group node window (S_w block width)
NVT = V // P        # 96 vtiles (group pairs)
GB = 8              # vtiles per gather batch
NB = NVT // GB      # 12 batches
SUP = 2048          # nodes per GEMM super-chunk
NSC = V // SUP      # 6 super-chunks
NCH = 24            # 512-node chunks for BN stats granularity

F32 = mybir.dt.float32
BF16 = mybir.dt.bfloat16
AF = mybir.ActivationFunctionType
BF = ml_dtypes.bfloat16

_cache = {}


def _build_schedule(edge_src, edge_dst, edge_weight):
    """Group edges by 64-node dst windows, pad each group to multiples of 128."""
    g_of_e = edge_dst // GSZ
    order = np.argsort(g_of_e, kind='stable')
    NG = V // GSZ
    counts = np.bincount(g_of_e, minlength=NG)
    sub_of_g = np.maximum(1, (counts + P - 1) // P)   # subtiles per group
    ST = int(sub_of_g.sum())
    idx_np = np.zeros((ST, P), np.int32)              # src per (subtile, lane)
    sw = np.zeros((ST, P, GSZ), np.float32)           # scatter blocks
    vt_subs = [[] for _ in range(NVT)]                # subtile ids per vtile half
    t = 0
    pos = 0
    for g in range(NG):
        eg = order[pos:pos + counts[g]]
        pos += counts[g]
        for s in range(sub_of_g[g]):
            part = eg[s * P:(s + 1) * P]
            n = len(part)
            idx_np[t, :n] = edge_src[part]
            sw[t, np.arange(n), edge_dst[part] - g * GSZ] = edge_weight[part]
            vt_subs[g // 2].append((t, g % 2))
            t += 1
    assert t == ST
    return idx_np, sw, vt_subs, ST


def _fold_weights(weight):
    # out = sum_k T_k(L) x W_k ; T0=I, T1=L, T2=2L^2-1, T3=4L^3-3L
    # power basis z_j = L^j x :  out = sum_j z_j Wf_j
    W = weight.astype(np.float64)
    Wf = np.stack([W[0] - W[2], W[1] - 3.0 * W[3], 2.0 * W[2], 4.0 * W[3]])
    # [(j,i), o] -> tiles [8, 128, 256]
    return Wf.reshape(K * FIN, FOUT).reshape(8, P, FOUT).astype(BF)


def _batches(vt_subs):
    """Per gather batch: (t0, nst, [(vt, [(s_local, h), ...]), ...])."""
    out = []
    for b in range(NB):
        vts = list(range(b * GB, (b + 1) * GB))
        t0 = vt_subs[vts[0]][0][0]
        nst = sum(len(vt_subs[vt]) for vt in vts)
        ventries = [(vt, [(t - t0, h) for (t, h) in vt_subs[vt]]) for vt in vts]
        out.append((t0, nst, ventries))
    return out


def _build_launch_a(ST, vt_subs):
    batches = _batches(vt_subs)
    nc = bacc.Bacc("TRN2", target_bir_lowering=False, debug=False, num_devices=8)
    xb = nc.dram_tensor("xb", [V, FIN], BF16, kind="ExternalInput").ap()
    # int16 gather indices, wrapped: idx k of the global edge-lane order
    # (k = subtile*128 + lane) lives at [k % 16, k // 16], replicated x8 rows.
    idx = nc.dram_tensor("idx", [P, ST * 8], mybir.dt.int16, kind="ExternalInput").ap()
    swt = nc.dram_tensor("swt", [P, ST * GSZ], BF16, kind="ExternalInput").ap()
    wf = nc.dram_tensor("wf", [8, P, FOUT], BF16, kind="ExternalInput").ap()
    rawT = nc.dram_tensor("rawT", [2, P, V], BF16, kind="ExternalOutput").ap()
    stats = nc.dram_tensor("stats", [P, 4], F32, kind="ExternalOutput").ap()
    zd = [xb] + [nc.dram_tensor(f"z{j}", [V, FIN], BF16).ap() for j in (1, 2, 3)]

    with tile.TileContext(nc) as tc, ExitStack() as ctx:
        cpool = ctx.enter_context(tc.tile_pool(name="const", bufs=1))
        idx_t = cpool.tile([P, ST * 8], mybir.dt.int16, tag="idx")
        nc.sync.dma_start(idx_t[:], idx[:, :])
        wf_t = cpool.tile([P, 8 * FOUT], BF16, tag="wf")
        nc.sync.dma_start(wf_t[:].rearrange("p (k o) -> p k o", k=8), wf.transpose([1, 0, 2]))

        # ---- Chebyshev (power-basis) recursion: z_j = L z_{j-1} ----
        with ExitStack() as rctx:
            swp = rctx.enter_context(tc.tile_pool(name="swp", bufs=3))
            msgp = rctx.enter_context(tc.tile_pool(name="msgp", bufs=2))
            outp = rctx.enter_context(tc.tile_pool(name="outp", bufs=3))
            psp = rctx.enter_context(tc.tile_pool(name="psp", bufs=8, space="PSUM"))
            for j in (1, 2, 3):
                for (t0, nst, ventries) in batches:
                    sw_t = swp.tile([P, nst * GSZ], BF16, tag="sw")
                    nc.sync.dma_start(sw_t[:], swt[:, t0 * GSZ:(t0 + nst) * GSZ])
                    msg_t = msgp.tile([P, nst * FIN], BF16, tag="msg")
                    nc.gpsimd.dma_gather(
                        out_ap=msg_t[:].rearrange("p (t f) -> p t f", t=nst),
                        in_ap=zd[j - 1][:, :],
                        idxs_ap=idx_t[:, t0 * 8:(t0 + nst) * 8],
                        num_idxs=nst * P,
                        num_idxs_reg=nst * P,
                        elem_size=FIN,
                        single_packet=False)
                    zo = outp.tile([P, GB * FIN], BF16, tag="zo")
                    for i, (vt, subs) in enumerate(ventries):
                        ps = psp.tile([P, FIN], F32, tag="acc")
                        half_count = [sum(1 for _, h in subs if h == hh) for hh in (0, 1)]
                        seen = [0, 0]
                        for (sl, h) in subs:
                            nc.tensor.matmul(
                                ps[h * GSZ:(h + 1) * GSZ, :],
                                sw_t[:, sl * GSZ:(sl + 1) * GSZ],
                                msg_t[:, sl * FIN:(sl + 1) * FIN],
                                start=(seen[h] == 0), stop=(seen[h] == half_count[h] - 1))
                            seen[h] += 1
                        if i % 2 == 0:
                            nc.scalar.activation(zo[:, i * FIN:(i + 1) * FIN], ps[:], AF.Copy)
                        else:
                            nc.vector.tensor_copy(zo[:, i * FIN:(i + 1) * FIN], ps[:])
                    bidx = t0  # unused; keep var for clarity
                    b = ventries[0][0] // GB
                    nc.sync.dma_start(
                        zd[j].rearrange("(b v p) f -> b p v f", v=GB, p=P)[b],
                        zo[:].rearrange("p (v f) -> p v f", v=GB))

        # ---- GEMM (zT via DMA-transpose) + BN stats ----
        with ExitStack() as gctx:
            ztp = gctx.enter_context(tc.tile_pool(name="ztp", bufs=2))
            rawp = gctx.enter_context(tc.tile_pool(name="rawp", bufs=2))
            psG = gctx.enter_context(tc.tile_pool(name="psG", bufs=4, space="PSUM"))
            sb = gctx.enter_context(tc.tile_pool(name="sb", bufs=1))
            stats_sb = sb.tile([P, 2 * NCH * 6], F32, tag="stats")
            for sc in range(NSC):
                zt = ztp.tile([P, 8 * SUP], BF16, tag="zt")
                for j in range(K):
                    for fh in range(2):
                        nc.sync.dma_start(
                            zt[:, (j * 2 + fh) * SUP:(j * 2 + fh + 1) * SUP],
                            zd[j][sc * SUP:(sc + 1) * SUP, fh * P:(fh + 1) * P],
                            transpose=True)
                raw_t = rawp.tile([P, 2 * SUP], BF16, tag="raw")
                for cc in range(SUP // 512):
                    for oh in range(2):
                        pg = psG.tile([P, 512], F32, tag="pg")
                        for kt in range(8):
                            nc.tensor.matmul(
                                pg[:], wf_t[:, kt * FOUT + oh * P: kt * FOUT + oh * P + P],
                                zt[:, kt * SUP + cc * 512: kt * SUP + (cc + 1) * 512],
                                start=(kt == 0), stop=(kt == 7))
                        c = sc * (SUP // 512) + cc
                        nc.vector.bn_stats(stats_sb[:, (oh * NCH + c) * 6:(oh * NCH + c) * 6 + 6], pg[:])
                        if (cc + oh) % 2 == 0:
                            nc.scalar.activation(raw_t[:, oh * SUP + cc * 512: oh * SUP + (cc + 1) * 512], pg[:], AF.Copy)
                        else:
                            nc.vector.tensor_copy(raw_t[:, oh * SUP + cc * 512: oh * SUP + (cc + 1) * 512], pg[:])
                for oh in range(2):
                    nc.sync.dma_start(rawT[oh][:, sc * SUP:(sc + 1) * SUP],
                                      raw_t[:, oh * SUP:(oh + 1) * SUP])
            aggr = sb.tile([P, 4], F32, tag="aggr")
            for oh in range(2):
                nc.vector.bn_aggr(aggr[:, oh * 2:oh * 2 + 2],
                                  stats_sb[:, oh * NCH * 6:(oh + 1) * NCH * 6])
            # stats out: [mean_h0, ex2_h0, mean_h1, ex2_h1]
            so = sb.tile([P, 4], F32, tag="so")
            for oh in range(2):
                m = aggr[:, oh * 2:oh * 2 + 1]
                v_ = aggr[:, oh * 2 + 1:oh * 2 + 2]
                nc.vector.tensor_copy(so[:, oh * 2:oh * 2 + 1], m)
                nc.vector.tensor_tensor(out=so[:, oh * 2 + 1:oh * 2 + 2], in0=m, in1=m,
                                        op=mybir.AluOpType.mult)
                nc.vector.tensor_tensor(out=so[:, oh * 2 + 1:oh * 2 + 2],
                                        in0=so[:, oh * 2 + 1:oh * 2 + 2], in1=v_,
                                        op=mybir.AluOpType.add)
            nc.sync.dma_start(stats[:, :], so[:])
    nc.compile()
    return nc


def _build_launch_b():
    nc = bacc.Bacc("TRN2", target_bir_lowering=False, debug=False, num_devices=8)
    rawT = nc.dram_tensor("rawT", [2, P, V], BF16, kind="ExternalInput").ap()
    sc = nc.dram_tensor("sc", [P, 2], F32, kind="ExternalInput").ap()
    sh = nc.dram_tensor("sh", [P, 2], F32, kind="ExternalInput").ap()
    out = nc.dram_tensor("out", [V, FOUT], F32, kind="ExternalOutput").ap()
    CH2 = 8           # vtiles per chunk
    NC2 = NVT // CH2  # 12 chunks
    from concourse.masks import make_identity
    with tile.TileContext(nc) as tc, ExitStack() as ctx:
        cpool = ctx.enter_context(tc.tile_pool(name="const", bufs=1))
        ident = cpool.tile([P, P], BF16, tag="id")
        make_identity(nc, ident[:])
        sc_t = cpool.tile([P, 2], F32, tag="sc")
        sh_t = cpool.tile([P, 2], F32, tag="sh")
        nc.sync.dma_start(sc_t[:], sc[:, :])
        nc.sync.dma_start(sh_t[:], sh[:, :])
        pool = ctx.enter_context(tc.tile_pool(name="sb", bufs=2))
        psp = ctx.enter_context(tc.tile_pool(name="ps", bufs=4, space="PSUM"))
        for c in range(NC2):
            nt = pool.tile([P, 2 * CH2 * P], BF16, tag="nt")
            for oh in range(2):
                nc.sync.dma_start(nt[:, oh * CH2 * P:(oh + 1) * CH2 * P],
                                  rawT[oh][:, c * CH2 * P:(c + 1) * CH2 * P])
            for oh in range(2):
                nc.scalar.activation(
                    nt[:, oh * CH2 * P:(oh + 1) * CH2 * P],
                    nt[:, oh * CH2 * P:(oh + 1) * CH2 * P],
                    AF.Relu, bias=sh_t[:, oh:oh + 1], scale=sc_t[:, oh:oh + 1])
            ot = pool.tile([P, CH2 * FOUT], F32, tag="ot")
            for vt in range(CH2):
                pt = psp.tile([P, FOUT], BF16, tag="pt")
                for oh in range(2):
                    nc.tensor.transpose(
                        pt[:, oh * P:(oh + 1) * P],
                        nt[:, oh * CH2 * P + vt * P: oh * CH2 * P + (vt + 1) * P],
                        ident[:])
                if vt % 2 == 0:
                    nc.vector.tensor_copy(ot[:, vt * FOUT:(vt + 1) * FOUT], pt[:])
                else:
                    nc.scalar.activation(ot[:, vt * FOUT:(vt + 1) * FOUT], pt[:], AF.Copy)
            nc.sync.dma_start(
                out.rearrange("(c vt p) f -> c p vt f", vt=CH2, p=P)[c],
                ot[:].rearrange("p (vt f) -> p vt f", vt=CH2))
    nc.compile()
    return nc


def kernel(x, edge_weight, weight, bias, gamma, beta, edge_src, edge_dst):
    x = np.asarray(x, np.float32)
    edge_weight = np.asarray(edge_weight, np.float32)
    weight = np.asarray(weight, np.float32)
    gamma = np.asarray(gamma, np.float32)
    beta = np.asarray(beta, np.float32)
    edge_src = np.asarray(edge_src, np.int32)
    edge_dst = np.asarray(edge_dst, np.int32)

    idx_np, sw, vt_subs, ST = _build_schedule(edge_src, edge_dst, edge_weight)
    key = ("A", ST, tuple(len(s) for s in vt_subs))
    if key not in _cache:
        _cache[key] = _build_launch_a(ST, vt_subs)
    ncA = _cache[key]
    if "B" not in _cache:
        _cache["B"] = _build_launch_b()
    ncB = _cache["B"]

    wf = _fold_weights(weight)
    swt = np.ascontiguousarray(sw.transpose(1, 0, 2)).reshape(P, ST * GSZ).astype(BF)
    # wrapped int16 gather indices (see launch A): k = t*128 + p
    idx_flat = idx_np.reshape(ST * P).astype(np.int16)
    idx16 = np.tile(idx_flat.reshape(-1, 16).T, (8, 1))  # [128, ST*8]
    idx16 = np.ascontiguousarray(idx16)
    in_maps = []
    for b in range(B):
        in_maps.append({
            "xb": np.ascontiguousarray(x[b]).astype(BF),
            "idx": idx16, "swt": swt, "wf": wf,
        })
    resA = run_bass_kernel_spmd(ncA, in_maps, core_ids=list(range(B)))

    # host: combine BN stats across cores (equal counts -> simple average)
    st = np.stack([np.asarray(resA.results[b]["stats"]) for b in range(B)])  # [B, 128, 4]
    mean = st[:, :, [0, 2]].mean(0)                               # [128, 2]
    ex2 = st[:, :, [1, 3]].mean(0)
    var = ex2 - mean * mean
    g2 = gamma.reshape(2, P).T                                    # [128, 2]
    b2 = beta.reshape(2, P).T
    scale = (g2 / np.sqrt(var + EPS)).astype(np.float32)
    shift = (b2 - mean * scale).astype(np.float32)

    in_maps_b = [{"rawT": resA.results[b]["rawT"], "sc": scale, "sh": shift}
                 for b in range(B)]
    resB = run_bass_kernel_spmd(ncB, in_maps_b, core_ids=list(range(B)))
    out = np.stack([np.asarray(resB.results[b]["out"]) for b in range(B)])
    # bias cancels inside training-mode BN (shifts the mean only); gamma/beta applied above
    return out.astype(np.float32)
